# revision 63
# baseline (speedup 1.0000x reference)
"""Trainium2 Bass kernel for multi-head attention decode step with KV cache.

Problem shapes (hardcoded): x[16,32,4096], wq/wk/wv/wo[4096,4096],
k_cache/v_cache[16,2048,32,128], start_pos=1024 (must be multiple of 128).

Sharding: tensor-parallel over the 32 heads -> 4 heads per core on 8 cores.
wq/wk/wv column-sharded, wo row-sharded; per-core partial outputs (full
[512, 4096] token-major tiles) summed on host.

Everything is streamed in bf16 (weights, x, KV caches, attention tensors);
PSUM accumulation stays fp32.

Per-core plan:
  Phase A: QT/KT = (w.T @ x.T) in [head_dim, token] layout, V in
           [token, feat] layout, via bf16 matmuls. RoPE applied on PSUM
           eviction using a deinterleaved head_dim permutation (evens then
           odds) so the rotate is a 64-partition swap.
  Phase B (b outer, h inner): per batch b all 4 heads' scoresT chunks
           (s on psum partitions), exp via ScalarE (scale=1/sqrt(128)),
           denominator via ones-vector matmuls into the same psum tile as
           the A.V results, 1/denom broadcast to 128 partitions on GpSimd
           (partition_broadcast), single fused normalize-evict multiply on
           DVE into the 4-head attnT tile. The 32 new tokens' scores/V use
           matmul tile_position to sit on partitions (b%4)*32 so V-new can
           be used as lhsT directly from the projection output (no copies).
           After each 4-batch group (128 tokens), wo is applied with the
           4 heads accumulated in PSUM, evicted bf16 and written to a
           token-major [512, 4096] output (partials summed on host).

Scheduling: K/V cache tiles use 7-deep slot rings (5 pool buffers + 2
retired x tiles each) so the streams prefetch through the wo bursts.
DMA queues: SP = first w chunks + K-cache + wo weights; Activation =
x + wq/wk/wv (all before any exp work starts) + exps + output chunks;
GpSimd = cos/sin + V-cache + partition broadcasts (vt emitted one batch
ahead of its broadcast so the normalize chain is never queued behind a
3.2us transfer).
"""

import numpy as np

B, T, D = 16, 32, 4096
HL, HD = 4, 128          # heads per core, head dim
NTOK = B * T             # 512
NC = 8

_STATE = {}


def _build(n_cached):
    import concourse.tile as tile
    from concourse import bacc, mybir
    from contextlib import ExitStack

    f32 = mybir.dt.float32
    bf16 = mybir.dt.bfloat16
    f8 = mybir.dt.float8e4

    SCF = n_cached // 128          # full cached s-chunks
    CW = SCF * 32 + 32             # scoresT free width (cached + new block)

    nc = bacc.Bacc("TRN2", target_bir_lowering=False, debug=False,
                   num_devices=NC)

    xt = nc.dram_tensor("xt", [D, NTOK], bf16, kind="ExternalInput").ap()
    wqd = nc.dram_tensor("wqd", [D, HL * HD], bf16, kind="ExternalInput").ap()
    wkd = nc.dram_tensor("wkd", [D, HL * HD], bf16, kind="ExternalInput").ap()
    wvd = nc.dram_tensor("wvd", [D, HL * HD], bf16, kind="ExternalInput").ap()
    wod = nc.dram_tensor("wod", [HL * HD, D], bf16, kind="ExternalInput").ap()
    ktc = nc.dram_tensor("ktc", [B, HL, HD, n_cached], bf16,
                         kind="ExternalInput").ap()
    vcc = nc.dram_tensor("vcc", [B, HL, 128, SCF * HD], bf16,
                         kind="ExternalInput").ap()
    cosd = nc.dram_tensor("cosd", [128, NTOK], f32, kind="ExternalInput").ap()
    sind = nc.dram_tensor("sind", [128, NTOK], f32, kind="ExternalInput").ap()
    outp = nc.dram_tensor("outp", [NTOK, D], bf16,
                          kind="ExternalOutput").ap()

    SCALE = float(1.0 / np.sqrt(np.float32(HD)))

    with tile.TileContext(nc) as tc:
        with ExitStack() as outer:
            st_pool = outer.enter_context(tc.tile_pool(name="st", bufs=1))
            dyn_pool = outer.enter_context(tc.tile_pool(name="dyn", bufs=2))
            kc_pool = outer.enter_context(tc.tile_pool(name="kc", bufs=1))
            vc_pool = outer.enter_context(tc.tile_pool(name="vc", bufs=1))
            pr_pool = outer.enter_context(tc.tile_pool(name="pr", bufs=6))
            obt_pool = outer.enter_context(tc.tile_pool(name="obt", bufs=4))

            cos_sb = st_pool.tile([128, NTOK], f32, tag="cos", name="cos_sb")
            sin_sb = st_pool.tile([128, NTOK], f32, tag="sin", name="sin_sb")
            nc.gpsimd.dma_start(cos_sb[:], cosd[:])
            nc.gpsimd.dma_start(sin_sb[:], sind[:])
            ones_col = st_pool.tile([128, 1], bf16, tag="oc", name="ones_col")
            nc.vector.memset(ones_col[:], 1.0)

            WO = [st_pool.tile([128, D], bf16, tag=f"wo{h}", name=f"WO{h}")
                  for h in range(HL)]

            QT = [st_pool.tile([128, NTOK], bf16, tag=f"q{m}", name=f"QT{m}")
                  for m in range(HL)]
            KT = [st_pool.tile([128, NTOK], bf16, tag=f"k{m}", name=f"KT{m}")
                  for m in range(HL)]
            VN = [st_pool.tile([128, HL * HD], bf16, tag=f"v{m}",
                               name=f"VN{m}") for m in range(4)]
            ATH = st_pool.tile([128, HL * NTOK], bf16, tag="ath", name="ATH")
            XT = [st_pool.tile([128, 4096], bf16, tag=f"x{t4}",
                               name=f"XT{t4}") for t4 in range(4)]
            rr_pool = rb_pool = ob_pool = rope_pool = dyn_pool

            # ---------------- Phase A: projections + rope ----------------
            with ExitStack() as pa:
                w_pool = pa.enter_context(tc.tile_pool(name="w", bufs=3))
                pp = pa.enter_context(
                    tc.tile_pool(name="pp", bufs=2, space="PSUM"))

                def load_x_half(t4, hf, chunks=1):
                    for c in range(chunks):
                        w = 512 // chunks
                        r0 = t4 * 1024 + hf * 512 + c * w
                        src = xt[r0:r0 + w, :].rearrange(
                            "(ks p) n -> p ks n", p=128)
                        c0 = hf * 2048 + c * w * 4
                        dst = XT[t4][:, c0:c0 + w * 4].rearrange(
                            "p (a b) -> p a b", a=w // 128)
                        nc.scalar.dma_start(dst, src)

                def load_w(wap, kc, chunks=1):
                    t = w_pool.tile([128, 2048], bf16, tag="w", name="wtile")
                    for c in range(chunks):
                        w = 512 // chunks
                        r0 = kc * 512 + c * w
                        src = wap[r0:r0 + w, :].rearrange(
                            "(ks p) n -> p ks n", p=128)
                        dst = t[:, c * w * 4:(c + 1) * w * 4].rearrange(
                            "p (a b) -> p a b", a=w // 128)
                        nc.scalar.dma_start(dst, src)
                    return t

                def rope_evict(ps, dst):
                    tsw = rope_pool.tile([128, NTOK], f32, tag="tsw",
                                         name="tsw")
                    nc.vector.tensor_copy(tsw[0:64, :], ps[64:128, :])
                    nc.vector.tensor_copy(tsw[64:128, :], ps[0:64, :])
                    tco = rope_pool.tile([128, NTOK], bf16, tag="tco",
                                         name="tco")
                    nc.vector.tensor_mul(tco[:], ps[:], cos_sb[:])
                    tsi = rope_pool.tile([128, NTOK], bf16, tag="tsi",
                                         name="tsi")
                    nc.vector.tensor_mul(tsi[:], tsw[:], sin_sb[:])
                    nc.vector.tensor_add(dst[:], tco[:], tsi[:])

                def mm_qk(ps, wt, ks, m, xtile, xoff, first, last):
                    # out[hd_m, tok] += w[:, m].T @ xT
                    nc.tensor.matmul(
                        ps[m][:],
                        wt[:, ks * 512 + m * 128:ks * 512 + (m + 1) * 128],
                        xtile[:, xoff:xoff + 512], start=first, stop=last)

                for wap, dsts in ((wqd, QT), (wkd, KT)):
                    ps = [pp.tile([128, 512], f32, tag=f"pp{m}",
                                  name=f"pp{m}") for m in range(4)]
                    for kc in range(8):
                        if wap is wqd and kc < 2:
                            # fine-grained first x/w loads, x on Activation
                            # and w on SP in parallel, so PE starts after ~2
                            # small DMAs and never starves in the ramp-up
                            wt = w_pool.tile([128, 2048], bf16, tag="w",
                                             name="wtile")
                            for c in range(4):
                                r = kc * 512 + c * 128
                                nc.scalar.dma_start(
                                    XT[0][:, kc * 2048 + c * 512:
                                          kc * 2048 + (c + 1) * 512],
                                    xt[r:r + 128, :])
                                nc.sync.dma_start(
                                    wt[:, c * 512:(c + 1) * 512],
                                    wqd[r:r + 128, :])
                        else:
                            if wap is wqd:
                                # interleave x halves with the w stream
                                load_x_half(kc // 2, kc % 2)
                            wt = load_w(wap, kc)
                        xtile = XT[kc // 2]
                        if kc < 7:
                            for ks in range(4):
                                first = (kc == 0 and ks == 0)
                                xoff = (kc % 2) * 2048 + ks * 512
                                for m in range(4):
                                    mm_qk(ps, wt, ks, m, xtile, xoff, first,
                                          False)
                        else:
                            # last k-chunk: m-major so each head's psum can
                            # be evicted while the next head still matmuls
                            for m in range(4):
                                for ks in range(4):
                                    xoff = (kc % 2) * 2048 + ks * 512
                                    mm_qk(ps, wt, ks, m, xtile, xoff, False,
                                          ks == 3)
                                rope_evict(ps[m], dsts[m])

                # V projection (bf16): out[tok_m, feat] += xT[:, m].T @ wv
                ps = [pp.tile([128, 512], f32, tag=f"pp{m}",
                              name=f"pp{m}") for m in range(4)]
                for kc in range(8):
                    wt = load_w(wvd, kc)
                    xtile = XT[kc // 2]
                    if kc < 7:
                        for ks in range(4):
                            xoff = (kc % 2) * 2048 + ks * 512
                            for m in range(4):
                                nc.tensor.matmul(
                                    ps[m][:],
                                    xtile[:, xoff + m * 128:
                                          xoff + (m + 1) * 128],
                                    wt[:, ks * 512:(ks + 1) * 512],
                                    start=(kc == 0 and ks == 0), stop=False)
                    else:
                        for m in range(4):
                            for ks in range(4):
                                xoff = (kc % 2) * 2048 + ks * 512
                                nc.tensor.matmul(
                                    ps[m][:],
                                    xtile[:, xoff + m * 128:
                                          xoff + (m + 1) * 128],
                                    wt[:, ks * 512:(ks + 1) * 512],
                                    start=False, stop=(ks == 3))
                            nc.vector.tensor_copy(VN[m][:], ps[m][:])

            # ------------- Phase B: attention + grouped wo ---------------
            with ExitStack() as pb:
                sp_ps = pb.enter_context(
                    tc.tile_pool(name="spps", bufs=2, space="PSUM"))
                av_ps = pb.enter_context(
                    tc.tile_pool(name="avps", bufs=2, space="PSUM"))
                po_ps = pb.enter_context(
                    tc.tile_pool(name="pops", bufs=2, space="PSUM"))

                pending = {}
                vts = {}

                # 7-deep K/V slot rings: 5 pool buffers + 2 retired XT tiles
                # (same [128, 4096] bf16 shape) per stream
                def kt_slot(b):
                    i = b % 7
                    if i < 5:
                        return kc_pool.tile([128, HL * n_cached], bf16,
                                            tag=f"kt{i}", name=f"kt{i}")
                    return XT[i - 5]

                def vt_slot(b):
                    i = b % 7
                    if i < 5:
                        return vc_pool.tile([128, HL * SCF * HD], bf16,
                                            tag=f"vt{i}", name=f"vt{i}")
                    return XT[2 + i - 5]

                def emit_vt(b):
                    vt_b = vt_slot(b)
                    nc.gpsimd.dma_start(
                        vt_b[:].rearrange("p (h s) -> p h s", h=HL),
                        vcc[b].rearrange("h p s -> p h s"))
                    vts[b] = vt_b

                def emit_b(b):
                    g, j = b // 4, b % 4
                    p0 = j * 32
                    kt_b = kt_slot(b)
                    nc.sync.dma_start(
                        kt_b[:].rearrange("p (h s) -> p h s", h=HL),
                        ktc[b].rearrange("h p s -> p h s"))
                    prs = []
                    # two heads per psum tile (bank-aligned halves) so one
                    # activation covers both, amortizing the access penalty
                    for hp in range(2):
                        sp = sp_ps.tile([128, 1024], f32, tag="sp",
                                        name="sp")
                        for hh in range(2):
                            h = hp * 2 + hh
                            off = hh * 512
                            qs = QT[h][:, b * 32:(b + 1) * 32]
                            for sc in range(SCF):
                                nc.tensor.matmul(
                                    sp[:, off + sc * 32:off + (sc + 1) * 32],
                                    kt_b[:, h * n_cached + sc * 128:
                                         h * n_cached + (sc + 1) * 128],
                                    qs, start=True, stop=True)
                            nc.tensor.matmul(
                                sp[p0:p0 + 32, off + SCF * 32:off + CW],
                                KT[h][:, b * 32:(b + 1) * 32], qs,
                                start=True, stop=True, tile_position=(0, p0))
                        pr = pr_pool.tile([128, 2 * CW], bf16, tag="pr",
                                          name="pr")
                        nc.scalar.activation(
                            pr[:].rearrange("p (i c) -> p i c", i=2),
                            sp[:].rearrange("p (i c) -> p i c",
                                            i=2)[:, :, 0:CW],
                            mybir.ActivationFunctionType.Exp, scale=SCALE)
                        prs.append(pr)
                    pending[b] = (prs, g, p0)

                def finish(b):
                    prs, g, p0 = pending.pop(b)
                    vt_b = vts.pop(b)
                    # av cols 0:128 = A.V for 4 heads; cols 128:256 row 0 =
                    # softmax denominators for 4 heads
                    av = av_ps.tile([128, 256], f32, tag="av", name="av")
                    for h in range(HL):
                        pr = prs[h // 2]
                        co = (h % 2) * CW
                        vo = h * SCF * HD
                        for sc in range(SCF):
                            nc.tensor.matmul(
                                av[:, h * 32:(h + 1) * 32],
                                vt_b[:, vo + sc * HD:vo + (sc + 1) * HD],
                                pr[:, co + sc * 32:co + (sc + 1) * 32],
                                start=(sc == 0), stop=False)
                        nc.tensor.matmul(
                            av[:, h * 32:(h + 1) * 32],
                            VN[g][p0:p0 + 32, h * HD:(h + 1) * HD],
                            pr[p0:p0 + 32, co + SCF * 32:co + CW],
                            start=False, stop=True, tile_position=(p0, 0))
                        dcol = 128 + h * 32
                        for sc in range(SCF):
                            nc.tensor.matmul(
                                av[0:1, dcol:dcol + 32], ones_col[:],
                                pr[:, co + sc * 32:co + (sc + 1) * 32],
                                start=(sc == 0), stop=False)
                        nc.tensor.matmul(
                            av[0:1, dcol:dcol + 32],
                            ones_col[p0:p0 + 32, :],
                            pr[p0:p0 + 32, co + SCF * 32:co + CW],
                            start=False, stop=True, tile_position=(p0, 0))
                    rr = rr_pool.tile([1, 128], f32, tag="rr", name="rr")
                    nc.vector.reciprocal(rr[:], av[0:1, 128:256])
                    rbs = rb_pool.tile([128, 128], f32, tag="rbs",
                                       name="rbs")
                    nc.gpsimd.partition_broadcast(rbs[:], rr[:])
                    # normalize + evict into the 4-head attnT tile
                    dst = ATH[:].rearrange(
                        "p (h t) -> p h t", h=HL)[:, :, b * 32:(b + 1) * 32]
                    src = av[:, 0:128].rearrange("p (h t) -> p h t", h=HL)
                    rbv = rbs[:].rearrange("p (h t) -> p h t", h=HL)
                    nc.vector.tensor_mul(dst, src, rbv)

                def emit_wo(g):
                    # last chunk of the last group in two half-tiles (own
                    # psum banks, so the bank-granular dep tracker doesn't
                    # serialize them) to shorten the final evict+DMA tail
                    chunks = [(oc * 512, (oc + 1) * 512) for oc in range(8)]
                    if g == 3:
                        chunks = chunks[:7] + [(3584, 3840), (3840, 4096)]
                    for lo, hi in chunks:
                        w = hi - lo
                        po = po_ps.tile([128, 512], f32, tag="po", name="po")
                        pool = obt_pool if g == 3 else ob_pool
                        ob = pool.tile([128, 512], bf16, tag="ob",
                                       name="ob")
                        for h in range(HL):
                            nc.tensor.matmul(
                                po[:, 0:w],
                                ATH[:, h * NTOK + g * 128:
                                    h * NTOK + (g + 1) * 128],
                                WO[h][:, lo:hi],
                                start=(h == 0), stop=(h == HL - 1))
                        nc.vector.tensor_copy(ob[:, 0:w], po[:, 0:w])
                        nc.scalar.dma_start(
                            outp[g * 128:(g + 1) * 128, lo:hi], ob[:, 0:w])

                for b in range(B):
                    if b == 5:
                        # wo weights on the SP queue after the first 5 kt
                        # transfers, before the XT-slot kt DMAs that block
                        # until the projections retire
                        for h in range(HL):
                            nc.sync.dma_start(WO[h][:],
                                              wod[h * 128:(h + 1) * 128, :])
                    emit_b(b)
                    if b == 0:
                        emit_vt(0)
                        emit_vt(1)
                    if b > 0:
                        # finish() ends with the partition_broadcast on the
                        # Pool queue; emit vt one batch ahead so the next
                        # broadcast is never stuck behind a cache transfer
                        finish(b - 1)
                        if b + 1 < B:
                            emit_vt(b + 1)
                    # wo for group g emitted 2 batches into group g+1 so the
                    # normalize chain of the group's last batch is long done
                    if b >= 6 and (b - 6) % 4 == 0:
                        emit_wo((b - 6) // 4)
                finish(B - 1)
                emit_wo(3)

    nc.compile()
    return nc


def _host_prep(x, wq, wk, wv, wo, k_cache, v_cache, n_cached):
    import ml_dtypes
    bf = ml_dtypes.bfloat16

    x = np.asarray(x, dtype=np.float32)
    wq = np.asarray(wq, dtype=np.float32)
    wk = np.asarray(wk, dtype=np.float32)
    wv = np.asarray(wv, dtype=np.float32)
    wo = np.asarray(wo, dtype=np.float32)
    k_cache = np.asarray(k_cache, dtype=np.float32)
    v_cache = np.asarray(v_cache, dtype=np.float32)

    SCF = n_cached // 128
    perm = np.concatenate([np.arange(0, HD, 2), np.arange(1, HD, 2)])

    xt = np.ascontiguousarray(x.reshape(NTOK, D).T.astype(bf))  # [D, NTOK]

    # rope tables in deinterleaved layout
    theta = (np.float32(10000.0) **
             (np.float32(-2.0) * np.arange(0, HD, 2, dtype=np.float32)
              / np.float32(HD)))                      # [64]
    freqs = np.arange(T, dtype=np.float32)[:, None] * theta[None, :]  # [T,64]
    cos_t = np.cos(freqs).astype(np.float32).T        # [64, T]
    sin_t = np.sin(freqs).astype(np.float32).T
    cos_rep = np.tile(cos_t, (1, B))                  # [64, NTOK]
    sin_rep = np.tile(sin_t, (1, B))
    cosd = np.ascontiguousarray(np.concatenate([cos_rep, cos_rep], axis=0))
    sind = np.ascontiguousarray(np.concatenate([-sin_rep, sin_rep], axis=0))

    in_maps = []
    for c in range(NC):
        hs = np.arange(c * HL, (c + 1) * HL)
        cols = (hs[:, None] * HD + perm[None, :]).reshape(-1)   # permuted q/k
        colsv = (hs[:, None] * HD + np.arange(HD)[None, :]).reshape(-1)
        wq_c = np.ascontiguousarray(wq[:, cols].astype(bf))
        wk_c = np.ascontiguousarray(wk[:, cols].astype(bf))
        wv_c = np.ascontiguousarray(wv[:, colsv].astype(bf))
        wo_c = np.ascontiguousarray(wo[colsv, :].astype(bf))
        # k cache: [b, h, hd(perm), s]
        kc_c = np.ascontiguousarray(
            k_cache[:, :n_cached][:, :, hs][:, :, :, perm]
            .transpose(0, 2, 3, 1).astype(bf))
        # v cache: [b, h, sp, sc, hd] -> flat [b, h, 128, SCF*HD]
        vc_c = np.ascontiguousarray(
            v_cache[:, :n_cached][:, :, hs]
            .reshape(B, SCF, 128, HL, HD)
            .transpose(0, 3, 2, 1, 4)
            .reshape(B, HL, 128, SCF * HD).astype(bf))
        in_maps.append({
            "xt": xt, "wqd": wq_c, "wkd": wk_c, "wvd": wv_c, "wod": wo_c,
            "ktc": kc_c, "vcc": vc_c, "cosd": cosd, "sind": sind,
        })
    return in_maps


def kernel(x, wq, wk, wv, wo, k_cache, v_cache, start_pos):
    from concourse import bass_utils

    n_cached = int(start_pos)
    assert n_cached % 128 == 0, "kernel assumes start_pos multiple of 128"

    if _STATE.get("n_cached") != n_cached:
        _STATE["nc"] = _build(n_cached)
        _STATE["n_cached"] = n_cached
    ncb = _STATE["nc"]

    in_maps = _host_prep(x, wq, wk, wv, wo, k_cache, v_cache, n_cached)
    res = None
    for attempt in range(4):
        try:
            res = bass_utils.run_bass_kernel_spmd(ncb, in_maps,
                                                  core_ids=list(range(NC)))
            break
        except Exception:
            if attempt == 3:
                raise
            import time as _time
            _time.sleep(20 * (attempt + 1))
    out = np.zeros((NTOK, D), dtype=np.float32)
    for c in range(NC):
        out += res.results[c]["outp"].astype(np.float32)
    return np.ascontiguousarray(out).reshape(B, T, D)


# revision 68
# speedup vs baseline: 1.0014x; 1.0014x over previous
"""Trainium2 Bass kernel for multi-head attention decode step with KV cache.

Problem shapes (hardcoded): x[16,32,4096], wq/wk/wv/wo[4096,4096],
k_cache/v_cache[16,2048,32,128], start_pos=1024 (must be multiple of 128).

Sharding: tensor-parallel over the 32 heads -> 4 heads per core on 8 cores.
wq/wk/wv column-sharded, wo row-sharded; per-core partial outputs (full
[512, 4096] token-major tiles) summed on host.

Everything is streamed in bf16 (weights, x, KV caches, attention tensors);
PSUM accumulation stays fp32.

Per-core plan:
  Phase A: QT/KT = (w.T @ x.T) in [head_dim, token] layout, V in
           [token, feat] layout, via bf16 matmuls. RoPE applied on PSUM
           eviction using a deinterleaved head_dim permutation (evens then
           odds) so the rotate is a 64-partition swap.
  Phase B (b outer, h inner): per batch b all 4 heads' scoresT chunks
           (s on psum partitions), exp via ScalarE (scale=1/sqrt(128)),
           denominator via ones-vector matmuls into the same psum tile as
           the A.V results, 1/denom broadcast to 128 partitions on GpSimd
           (partition_broadcast), single fused normalize-evict multiply on
           DVE into the 4-head attnT tile. The 32 new tokens' scores/V use
           matmul tile_position to sit on partitions (b%4)*32 so V-new can
           be used as lhsT directly from the projection output (no copies).
           After each 4-batch group (128 tokens), wo is applied with the
           4 heads accumulated in PSUM, evicted bf16 and written to a
           token-major [512, 4096] output (partials summed on host).

Scheduling: K/V cache tiles use 7-deep slot rings (5 pool buffers + 2
retired x tiles each) so the streams prefetch through the wo bursts.
DMA queues: SP = first w chunks + K-cache + wo weights; Activation =
x + wq/wk/wv (all before any exp work starts) + exps + output chunks;
GpSimd = cos/sin + V-cache + partition broadcasts (vt emitted one batch
ahead of its broadcast so the normalize chain is never queued behind a
3.2us transfer).
"""

import numpy as np

B, T, D = 16, 32, 4096
HL, HD = 4, 128          # heads per core, head dim
NTOK = B * T             # 512
NC = 8

_STATE = {}


def _build(n_cached):
    import concourse.tile as tile
    from concourse import bacc, mybir
    from contextlib import ExitStack

    f32 = mybir.dt.float32
    bf16 = mybir.dt.bfloat16
    f8 = mybir.dt.float8e4

    SCF = n_cached // 128          # full cached s-chunks
    CW = SCF * 32 + 32             # scoresT free width (cached + new block)

    nc = bacc.Bacc("TRN2", target_bir_lowering=False, debug=False,
                   num_devices=NC)

    xt = nc.dram_tensor("xt", [D, NTOK], bf16, kind="ExternalInput").ap()
    wqd = nc.dram_tensor("wqd", [D, HL * HD], bf16, kind="ExternalInput").ap()
    wkd = nc.dram_tensor("wkd", [D, HL * HD], bf16, kind="ExternalInput").ap()
    wvd = nc.dram_tensor("wvd", [D, HL * HD], bf16, kind="ExternalInput").ap()
    wod = nc.dram_tensor("wod", [HL * HD, D], bf16, kind="ExternalInput").ap()
    ktc = nc.dram_tensor("ktc", [B, HL, HD, n_cached], bf16,
                         kind="ExternalInput").ap()
    vcc = nc.dram_tensor("vcc", [B, HL, 128, SCF * HD], bf16,
                         kind="ExternalInput").ap()
    iden = nc.dram_tensor("iden", [128, 128], f32,
                          kind="ExternalInput").ap()
    cosd = nc.dram_tensor("cosd", [128, NTOK], f32, kind="ExternalInput").ap()
    sind = nc.dram_tensor("sind", [128, NTOK], f32, kind="ExternalInput").ap()
    outp = nc.dram_tensor("outp", [NTOK, D], bf16,
                          kind="ExternalOutput").ap()

    SCALE = float(1.0 / np.sqrt(np.float32(HD)))

    with tile.TileContext(nc) as tc:
        with ExitStack() as outer:
            st_pool = outer.enter_context(tc.tile_pool(name="st", bufs=1))
            dyn_pool = outer.enter_context(tc.tile_pool(name="dyn", bufs=2))
            kc_pool = outer.enter_context(tc.tile_pool(name="kc", bufs=1))
            vc_pool = outer.enter_context(tc.tile_pool(name="vc", bufs=1))
            pr_pool = outer.enter_context(tc.tile_pool(name="pr", bufs=6))
            obt_pool = outer.enter_context(tc.tile_pool(name="obt", bufs=4))

            cos_sb = st_pool.tile([128, NTOK], f32, tag="cos", name="cos_sb")
            sin_sb = st_pool.tile([128, NTOK], f32, tag="sin", name="sin_sb")
            nc.gpsimd.dma_start(cos_sb[:], cosd[:])
            IDEN = st_pool.tile([128, 128], f32, tag="iden", name="IDEN")
            nc.gpsimd.dma_start(IDEN[:], iden[:])
            nc.gpsimd.dma_start(sin_sb[:], sind[:])
            ones_col = st_pool.tile([128, 1], bf16, tag="oc", name="ones_col")
            nc.vector.memset(ones_col[:], 1.0)

            WO = [st_pool.tile([128, D], bf16, tag=f"wo{h}", name=f"WO{h}")
                  for h in range(HL)]

            QT = [st_pool.tile([128, NTOK], bf16, tag=f"q{m}", name=f"QT{m}")
                  for m in range(HL)]
            KT = [st_pool.tile([128, NTOK], bf16, tag=f"k{m}", name=f"KT{m}")
                  for m in range(HL)]
            VN = [st_pool.tile([128, HL * HD], bf16, tag=f"v{m}",
                               name=f"VN{m}") for m in range(4)]
            ATH = st_pool.tile([128, HL * NTOK], bf16, tag="ath", name="ATH")
            XT = [st_pool.tile([128, 4096], bf16, tag=f"x{t4}",
                               name=f"XT{t4}") for t4 in range(4)]
            rr_pool = rb_pool = ob_pool = rope_pool = dyn_pool

            # ---------------- Phase A: projections + rope ----------------
            with ExitStack() as pa:
                w_pool = pa.enter_context(tc.tile_pool(name="w", bufs=3))
                pp = pa.enter_context(
                    tc.tile_pool(name="pp", bufs=2, space="PSUM"))

                def load_x_half(t4, hf, chunks=1):
                    for c in range(chunks):
                        w = 512 // chunks
                        r0 = t4 * 1024 + hf * 512 + c * w
                        src = xt[r0:r0 + w, :].rearrange(
                            "(ks p) n -> p ks n", p=128)
                        c0 = hf * 2048 + c * w * 4
                        dst = XT[t4][:, c0:c0 + w * 4].rearrange(
                            "p (a b) -> p a b", a=w // 128)
                        nc.scalar.dma_start(dst, src)

                def load_w(wap, kc, chunks=1):
                    t = w_pool.tile([128, 2048], bf16, tag="w", name="wtile")
                    for c in range(chunks):
                        w = 512 // chunks
                        r0 = kc * 512 + c * w
                        src = wap[r0:r0 + w, :].rearrange(
                            "(ks p) n -> p ks n", p=128)
                        dst = t[:, c * w * 4:(c + 1) * w * 4].rearrange(
                            "p (a b) -> p a b", a=w // 128)
                        nc.scalar.dma_start(dst, src)
                    return t

                def rope_evict(ps, dst):
                    tsw = rope_pool.tile([128, NTOK], f32, tag="tsw",
                                         name="tsw")
                    nc.vector.tensor_copy(tsw[0:64, :], ps[64:128, :])
                    nc.vector.tensor_copy(tsw[64:128, :], ps[0:64, :])
                    tco = rope_pool.tile([128, NTOK], bf16, tag="tco",
                                         name="tco")
                    nc.vector.tensor_mul(tco[:], ps[:], cos_sb[:])
                    tsi = rope_pool.tile([128, NTOK], bf16, tag="tsi",
                                         name="tsi")
                    nc.vector.tensor_mul(tsi[:], tsw[:], sin_sb[:])
                    nc.vector.tensor_add(dst[:], tco[:], tsi[:])

                def mm_qk(ps, wt, ks, m, xtile, xoff, first, last):
                    # out[hd_m, tok] += w[:, m].T @ xT
                    nc.tensor.matmul(
                        ps[m][:],
                        wt[:, ks * 512 + m * 128:ks * 512 + (m + 1) * 128],
                        xtile[:, xoff:xoff + 512], start=first, stop=last)

                for wap, dsts in ((wqd, QT), (wkd, KT)):
                    ps = [pp.tile([128, 512], f32, tag=f"pp{m}",
                                  name=f"pp{m}") for m in range(4)]
                    for kc in range(8):
                        if wap is wqd and kc < 2:
                            # fine-grained first x/w loads, x on Activation
                            # and w on SP in parallel, so PE starts after ~2
                            # small DMAs and never starves in the ramp-up
                            wt = w_pool.tile([128, 2048], bf16, tag="w",
                                             name="wtile")
                            for c in range(4):
                                r = kc * 512 + c * 128
                                nc.scalar.dma_start(
                                    XT[0][:, kc * 2048 + c * 512:
                                          kc * 2048 + (c + 1) * 512],
                                    xt[r:r + 128, :])
                                nc.sync.dma_start(
                                    wt[:, c * 512:(c + 1) * 512],
                                    wqd[r:r + 128, :])
                        else:
                            if wap is wqd:
                                # interleave x halves with the w stream
                                load_x_half(kc // 2, kc % 2)
                            wt = load_w(wap, kc)
                        xtile = XT[kc // 2]
                        if kc < 7:
                            for ks in range(4):
                                first = (kc == 0 and ks == 0)
                                xoff = (kc % 2) * 2048 + ks * 512
                                for m in range(4):
                                    mm_qk(ps, wt, ks, m, xtile, xoff, first,
                                          False)
                        else:
                            # last k-chunk: m-major so each head's psum can
                            # be evicted while the next head still matmuls
                            for m in range(4):
                                for ks in range(4):
                                    xoff = (kc % 2) * 2048 + ks * 512
                                    mm_qk(ps, wt, ks, m, xtile, xoff, False,
                                          ks == 3)
                                rope_evict(ps[m], dsts[m])

                # V projection (bf16): out[tok_m, feat] += xT[:, m].T @ wv
                ps = [pp.tile([128, 512], f32, tag=f"pp{m}",
                              name=f"pp{m}") for m in range(4)]
                for kc in range(8):
                    wt = load_w(wvd, kc)
                    xtile = XT[kc // 2]
                    if kc < 7:
                        for ks in range(4):
                            xoff = (kc % 2) * 2048 + ks * 512
                            for m in range(4):
                                nc.tensor.matmul(
                                    ps[m][:],
                                    xtile[:, xoff + m * 128:
                                          xoff + (m + 1) * 128],
                                    wt[:, ks * 512:(ks + 1) * 512],
                                    start=(kc == 0 and ks == 0), stop=False)
                    else:
                        for m in range(4):
                            for ks in range(4):
                                xoff = (kc % 2) * 2048 + ks * 512
                                nc.tensor.matmul(
                                    ps[m][:],
                                    xtile[:, xoff + m * 128:
                                          xoff + (m + 1) * 128],
                                    wt[:, ks * 512:(ks + 1) * 512],
                                    start=False, stop=(ks == 3))
                            nc.vector.tensor_copy(VN[m][:], ps[m][:])

            # ------------- Phase B: attention + grouped wo ---------------
            with ExitStack() as pb:
                sp_ps = pb.enter_context(
                    tc.tile_pool(name="spps", bufs=2, space="PSUM"))
                av_ps = pb.enter_context(
                    tc.tile_pool(name="avps", bufs=2, space="PSUM"))
                po_ps = pb.enter_context(
                    tc.tile_pool(name="pops", bufs=2, space="PSUM"))

                pending = {}
                vts = {}

                # 7-deep K/V slot rings: 5 pool buffers + 2 retired XT tiles
                # (same [128, 4096] bf16 shape) per stream
                def kt_slot(b):
                    i = b % 7
                    if i < 5:
                        return kc_pool.tile([128, HL * n_cached], bf16,
                                            tag=f"kt{i}", name=f"kt{i}")
                    return XT[i - 5]

                def vt_slot(b):
                    i = b % 7
                    if i < 5:
                        return vc_pool.tile([128, HL * SCF * HD], bf16,
                                            tag=f"vt{i}", name=f"vt{i}")
                    return XT[2 + i - 5]

                def emit_vt(b):
                    vt_b = vt_slot(b)
                    eng = nc.gpsimd
                    eng.dma_start(
                        vt_b[:].rearrange("p (h s) -> p h s", h=HL),
                        vcc[b].rearrange("h p s -> p h s"))
                    vts[b] = vt_b

                kts = {}

                def emit_kt(b, eng):
                    kt_b = kt_slot(b)
                    eng.dma_start(
                        kt_b[:].rearrange("p (h s) -> p h s", h=HL),
                        ktc[b].rearrange("h p s -> p h s"))
                    kts[b] = kt_b

                def emit_b(b):
                    g, j = b // 4, b % 4
                    p0 = j * 32
                    if b not in kts:
                        emit_kt(b, nc.sync)
                    kt_b = kts.pop(b)
                    prs = []
                    # two heads per psum tile (bank-aligned halves) so one
                    # activation covers both, amortizing the access penalty
                    for hp in range(2):
                        sp = sp_ps.tile([128, 1024], f32, tag="sp",
                                        name="sp")
                        for hh in range(2):
                            h = hp * 2 + hh
                            off = hh * 512
                            qs = QT[h][:, b * 32:(b + 1) * 32]
                            for sc in range(SCF):
                                nc.tensor.matmul(
                                    sp[:, off + sc * 32:off + (sc + 1) * 32],
                                    kt_b[:, h * n_cached + sc * 128:
                                         h * n_cached + (sc + 1) * 128],
                                    qs, start=True, stop=True)
                            nc.tensor.matmul(
                                sp[p0:p0 + 32, off + SCF * 32:off + CW],
                                KT[h][:, b * 32:(b + 1) * 32], qs,
                                start=True, stop=True, tile_position=(0, p0))
                        pr = pr_pool.tile([128, 2 * CW], bf16, tag="pr",
                                          name="pr")
                        nc.scalar.activation(
                            pr[:].rearrange("p (i c) -> p i c", i=2),
                            sp[:].rearrange("p (i c) -> p i c",
                                            i=2)[:, :, 0:CW],
                            mybir.ActivationFunctionType.Exp, scale=SCALE)
                        prs.append(pr)
                    pending[b] = (prs, g, p0)

                normq = {}

                def finish(b):
                    prs, g, p0 = pending.pop(b)
                    vt_b = vts.pop(b)
                    # av cols 0:128 = A.V for 4 heads; col 128 = transposed
                    # softmax denominators (token-on-partition, free size 1
                    # per accumulation step); row 0 cols 256:384 later holds
                    # the transposed reciprocals
                    av = av_ps.tile([128, 384], f32, tag="av", name="av")
                    for h in range(HL):
                        pr = prs[h // 2]
                        co = (h % 2) * CW
                        vo = h * SCF * HD
                        for sc in range(SCF):
                            nc.tensor.matmul(
                                av[:, h * 32:(h + 1) * 32],
                                vt_b[:, vo + sc * HD:vo + (sc + 1) * HD],
                                pr[:, co + sc * 32:co + (sc + 1) * 32],
                                start=(sc == 0), stop=False)
                        nc.tensor.matmul(
                            av[:, h * 32:(h + 1) * 32],
                            VN[g][p0:p0 + 32, h * HD:(h + 1) * HD],
                            pr[p0:p0 + 32, co + SCF * 32:co + CW],
                            start=False, stop=True, tile_position=(p0, 0))
                        for sc in range(SCF):
                            nc.tensor.matmul(
                                av[h * 32:(h + 1) * 32, 128:129],
                                pr[:, co + sc * 32:co + (sc + 1) * 32],
                                ones_col[:],
                                start=(sc == 0), stop=False,
                                tile_position=(0, h * 32))
                        nc.tensor.matmul(
                            av[h * 32:(h + 1) * 32, 128:129],
                            pr[p0:p0 + 32, co + SCF * 32:co + CW],
                            ones_col[p0:p0 + 32, :],
                            start=False, stop=True,
                            tile_position=(p0, h * 32))
                    rrc = rr_pool.tile([128, 1], f32, tag="rrc", name="rrc")
                    nc.vector.reciprocal(rrc[:], av[:, 128:129])
                    normq[b] = (av, rrc)

                def norm2(b):
                    # deferred 2 iterations: transpose the reciprocals to a
                    # row, broadcast on GpSimd, fused normalize-evict on DVE
                    av, rrc = normq.pop(b)
                    nc.tensor.matmul(av[0:1, 256:384], rrc[:], IDEN[:],
                                     is_transpose=True)
                    rrs = rr_pool.tile([1, 128], f32, tag="rrs", name="rrs")
                    nc.vector.tensor_copy(rrs[:], av[0:1, 256:384])
                    rbs = rb_pool.tile([128, 128], f32, tag="rbs",
                                       name="rbs")
                    nc.gpsimd.partition_broadcast(rbs[:], rrs[:])
                    dst = ATH[:].rearrange(
                        "p (h t) -> p h t", h=HL)[:, :, b * 32:(b + 1) * 32]
                    src = av[:, 0:128].rearrange("p (h t) -> p h t", h=HL)
                    rbv = rbs[:].rearrange("p (h t) -> p h t", h=HL)
                    nc.vector.tensor_mul(dst, src, rbv)

                def emit_wo(g):
                    # last chunk of the last group in two half-tiles (own
                    # psum banks, so the bank-granular dep tracker doesn't
                    # serialize them) to shorten the final evict+DMA tail
                    chunks = [(oc * 512, (oc + 1) * 512) for oc in range(8)]
                    if g == 3:
                        chunks = chunks[:7] + [(3584, 3840), (3840, 4096)]
                    for lo, hi in chunks:
                        w = hi - lo
                        po = po_ps.tile([128, 512], f32, tag="po", name="po")
                        pool = obt_pool if g == 3 else ob_pool
                        ob = pool.tile([128, 512], bf16, tag="ob",
                                       name="ob")
                        for h in range(HL):
                            nc.tensor.matmul(
                                po[:, 0:w],
                                ATH[:, h * NTOK + g * 128:
                                    h * NTOK + (g + 1) * 128],
                                WO[h][:, lo:hi],
                                start=(h == 0), stop=(h == HL - 1))
                        nc.vector.tensor_copy(ob[:, 0:w], po[:, 0:w])
                        nc.scalar.dma_start(
                            outp[g * 128:(g + 1) * 128, lo:hi], ob[:, 0:w])

                for b in range(B):
                    if b == 5:
                        # wo weights on the SP queue after the first 5 kt
                        # transfers, before the XT-slot kt DMAs that block
                        # until the projections retire
                        for h in range(HL):
                            nc.sync.dma_start(WO[h][:],
                                              wod[h * 128:(h + 1) * 128, :])
                    emit_b(b)
                    if b >= 2:
                        norm2(b - 2)
                    if b == 0:
                        emit_vt(0)
                        emit_vt(1)
                    if b > 0:
                        # norm2() ends with the partition_broadcast on the
                        # Pool queue; emit vt after it so the broadcast is
                        # never stuck behind a cache transfer
                        finish(b - 1)
                        if b + 1 < B:
                            emit_vt(b + 1)
                    # wo for group g emitted 2 batches into group g+1 so the
                    # normalize chain of the group's last batch is long done
                    if b >= 6 and (b - 6) % 4 == 0:
                        emit_wo((b - 6) // 4)
                finish(B - 1)
                norm2(B - 2)
                norm2(B - 1)
                emit_wo(3)

    nc.compile()
    return nc


def _host_prep(x, wq, wk, wv, wo, k_cache, v_cache, n_cached):
    import ml_dtypes
    bf = ml_dtypes.bfloat16

    x = np.asarray(x, dtype=np.float32)
    wq = np.asarray(wq, dtype=np.float32)
    wk = np.asarray(wk, dtype=np.float32)
    wv = np.asarray(wv, dtype=np.float32)
    wo = np.asarray(wo, dtype=np.float32)
    k_cache = np.asarray(k_cache, dtype=np.float32)
    v_cache = np.asarray(v_cache, dtype=np.float32)

    SCF = n_cached // 128
    perm = np.concatenate([np.arange(0, HD, 2), np.arange(1, HD, 2)])

    xt = np.ascontiguousarray(x.reshape(NTOK, D).T.astype(bf))  # [D, NTOK]

    # rope tables in deinterleaved layout
    theta = (np.float32(10000.0) **
             (np.float32(-2.0) * np.arange(0, HD, 2, dtype=np.float32)
              / np.float32(HD)))                      # [64]
    freqs = np.arange(T, dtype=np.float32)[:, None] * theta[None, :]  # [T,64]
    cos_t = np.cos(freqs).astype(np.float32).T        # [64, T]
    sin_t = np.sin(freqs).astype(np.float32).T
    iden = np.ascontiguousarray(np.eye(128, dtype=np.float32))
    cos_rep = np.tile(cos_t, (1, B))                  # [64, NTOK]
    sin_rep = np.tile(sin_t, (1, B))
    cosd = np.ascontiguousarray(np.concatenate([cos_rep, cos_rep], axis=0))
    sind = np.ascontiguousarray(np.concatenate([-sin_rep, sin_rep], axis=0))

    in_maps = []
    for c in range(NC):
        hs = np.arange(c * HL, (c + 1) * HL)
        cols = (hs[:, None] * HD + perm[None, :]).reshape(-1)   # permuted q/k
        colsv = (hs[:, None] * HD + np.arange(HD)[None, :]).reshape(-1)
        wq_c = np.ascontiguousarray(wq[:, cols].astype(bf))
        wk_c = np.ascontiguousarray(wk[:, cols].astype(bf))
        wv_c = np.ascontiguousarray(wv[:, colsv].astype(bf))
        wo_c = np.ascontiguousarray(wo[colsv, :].astype(bf))
        # k cache: [b, h, hd(perm), s]
        kc_c = np.ascontiguousarray(
            k_cache[:, :n_cached][:, :, hs][:, :, :, perm]
            .transpose(0, 2, 3, 1).astype(bf))
        # v cache: [b, h, sp, sc, hd] -> flat [b, h, 128, SCF*HD]
        vc_c = np.ascontiguousarray(
            v_cache[:, :n_cached][:, :, hs]
            .reshape(B, SCF, 128, HL, HD)
            .transpose(0, 3, 2, 1, 4)
            .reshape(B, HL, 128, SCF * HD).astype(bf))
        in_maps.append({
            "xt": xt, "wqd": wq_c, "wkd": wk_c, "wvd": wv_c, "wod": wo_c,
            "ktc": kc_c, "vcc": vc_c, "cosd": cosd, "sind": sind,
            "iden": iden,
        })
    return in_maps


def kernel(x, wq, wk, wv, wo, k_cache, v_cache, start_pos):
    from concourse import bass_utils

    n_cached = int(start_pos)
    assert n_cached % 128 == 0, "kernel assumes start_pos multiple of 128"

    if _STATE.get("n_cached") != n_cached:
        _STATE["nc"] = _build(n_cached)
        _STATE["n_cached"] = n_cached
    ncb = _STATE["nc"]

    in_maps = _host_prep(x, wq, wk, wv, wo, k_cache, v_cache, n_cached)
    res = None
    for attempt in range(4):
        try:
            res = bass_utils.run_bass_kernel_spmd(ncb, in_maps,
                                                  core_ids=list(range(NC)))
            break
        except Exception:
            if attempt == 3:
                raise
            import time as _time
            _time.sleep(20 * (attempt + 1))
    out = np.zeros((NTOK, D), dtype=np.float32)
    for c in range(NC):
        out += res.results[c]["outp"].astype(np.float32)
    return np.ascontiguousarray(out).reshape(B, T, D)


# revision 71
# speedup vs baseline: 1.0021x; 1.0007x over previous
"""Trainium2 Bass kernel for multi-head attention decode step with KV cache.

Problem shapes (hardcoded): x[16,32,4096], wq/wk/wv/wo[4096,4096],
k_cache/v_cache[16,2048,32,128], start_pos=1024 (must be multiple of 128).

Sharding: tensor-parallel over the 32 heads -> 4 heads per core on 8 cores.
wq/wk/wv column-sharded, wo row-sharded; per-core partial outputs (full
[512, 4096] token-major tiles) summed on host.

Everything is streamed in bf16 (weights, x, KV caches, attention tensors);
PSUM accumulation stays fp32.

Per-core plan:
  Phase A: QT/KT = (w.T @ x.T) in [head_dim, token] layout, V in
           [token, feat] layout, via bf16 matmuls. RoPE applied on PSUM
           eviction using a deinterleaved head_dim permutation (evens then
           odds) so the rotate is a 64-partition swap.
  Phase B (b outer, h inner): per batch b all 4 heads' scoresT chunks
           (s on psum partitions), exp via ScalarE (scale=1/sqrt(128)),
           denominator via ones-vector matmuls into the same psum tile as
           the A.V results, 1/denom broadcast to 128 partitions on GpSimd
           (partition_broadcast), single fused normalize-evict multiply on
           DVE into the 4-head attnT tile. The 32 new tokens' scores/V use
           matmul tile_position to sit on partitions (b%4)*32 so V-new can
           be used as lhsT directly from the projection output (no copies).
           After each 4-batch group (128 tokens), wo is applied with the
           4 heads accumulated in PSUM, evicted bf16 and written to a
           token-major [512, 4096] output (partials summed on host).

Scheduling: K/V cache tiles use 7-deep slot rings (5 pool buffers + 2
retired x tiles each) so the streams prefetch through the wo bursts.
DMA queues: SP = first w chunks + K-cache + wo weights; Activation =
x + wq/wk/wv (all before any exp work starts) + exps + output chunks;
GpSimd = cos/sin + V-cache + partition broadcasts (vt emitted one batch
ahead of its broadcast so the normalize chain is never queued behind a
3.2us transfer).
"""

import numpy as np

B, T, D = 16, 32, 4096
HL, HD = 4, 128          # heads per core, head dim
NTOK = B * T             # 512
NC = 8

_STATE = {}


def _build(n_cached):
    import concourse.tile as tile
    from concourse import bacc, mybir
    from contextlib import ExitStack

    f32 = mybir.dt.float32
    bf16 = mybir.dt.bfloat16
    f8 = mybir.dt.float8e4

    SCF = n_cached // 128          # full cached s-chunks
    CW = SCF * 32 + 32             # scoresT free width (cached + new block)

    nc = bacc.Bacc("TRN2", target_bir_lowering=False, debug=False,
                   num_devices=NC)

    xt = nc.dram_tensor("xt", [D, NTOK], bf16, kind="ExternalInput").ap()
    wqd = nc.dram_tensor("wqd", [D, HL * HD], bf16, kind="ExternalInput").ap()
    wkd = nc.dram_tensor("wkd", [D, HL * HD], bf16, kind="ExternalInput").ap()
    wvd = nc.dram_tensor("wvd", [D, HL * HD], bf16, kind="ExternalInput").ap()
    wod = nc.dram_tensor("wod", [HL * HD, D], bf16, kind="ExternalInput").ap()
    ktc = nc.dram_tensor("ktc", [B, HL, HD, n_cached], bf16,
                         kind="ExternalInput").ap()
    vcc = nc.dram_tensor("vcc", [B, HL, 128, SCF * HD], bf16,
                         kind="ExternalInput").ap()
    iden = nc.dram_tensor("iden", [128, 128], f32,
                          kind="ExternalInput").ap()
    cosd = nc.dram_tensor("cosd", [128, NTOK], f32, kind="ExternalInput").ap()
    sind = nc.dram_tensor("sind", [128, NTOK], f32, kind="ExternalInput").ap()
    outp = nc.dram_tensor("outp", [NTOK, D], bf16,
                          kind="ExternalOutput").ap()

    SCALE = float(1.0 / np.sqrt(np.float32(HD)))

    with tile.TileContext(nc) as tc:
        with ExitStack() as outer:
            st_pool = outer.enter_context(tc.tile_pool(name="st", bufs=1))
            dyn_pool = outer.enter_context(tc.tile_pool(name="dyn", bufs=2))
            kc_pool = outer.enter_context(tc.tile_pool(name="kc", bufs=1))
            vc_pool = outer.enter_context(tc.tile_pool(name="vc", bufs=1))
            pr_pool = outer.enter_context(tc.tile_pool(name="pr", bufs=6))
            obt_pool = outer.enter_context(tc.tile_pool(name="obt", bufs=4))

            cos_sb = st_pool.tile([128, NTOK], f32, tag="cos", name="cos_sb")
            sin_sb = st_pool.tile([128, NTOK], f32, tag="sin", name="sin_sb")
            nc.gpsimd.dma_start(cos_sb[:], cosd[:])
            IDEN = st_pool.tile([128, 128], f32, tag="iden", name="IDEN")
            nc.gpsimd.dma_start(IDEN[:], iden[:])
            nc.gpsimd.dma_start(sin_sb[:], sind[:])
            ones_col = st_pool.tile([128, 1], bf16, tag="oc", name="ones_col")
            nc.vector.memset(ones_col[:], 1.0)

            WO = [st_pool.tile([128, D], bf16, tag=f"wo{h}", name=f"WO{h}")
                  for h in range(HL)]

            QT = [st_pool.tile([128, NTOK], bf16, tag=f"q{m}", name=f"QT{m}")
                  for m in range(HL)]
            KT = [st_pool.tile([128, NTOK], bf16, tag=f"k{m}", name=f"KT{m}")
                  for m in range(HL)]
            VN = [st_pool.tile([128, HL * HD], bf16, tag=f"v{m}",
                               name=f"VN{m}") for m in range(4)]
            ATH = st_pool.tile([128, HL * NTOK], bf16, tag="ath", name="ATH")
            XT = [st_pool.tile([128, 4096], bf16, tag=f"x{t4}",
                               name=f"XT{t4}") for t4 in range(4)]
            rr_pool = rb_pool = ob_pool = rope_pool = dyn_pool

            # ---------------- Phase A: projections + rope ----------------
            with ExitStack() as pa:
                w_pool = pa.enter_context(tc.tile_pool(name="w", bufs=3))
                pp = pa.enter_context(
                    tc.tile_pool(name="pp", bufs=2, space="PSUM"))

                def load_x_half(t4, hf, chunks=1):
                    for c in range(chunks):
                        w = 512 // chunks
                        r0 = t4 * 1024 + hf * 512 + c * w
                        src = xt[r0:r0 + w, :].rearrange(
                            "(ks p) n -> p ks n", p=128)
                        c0 = hf * 2048 + c * w * 4
                        dst = XT[t4][:, c0:c0 + w * 4].rearrange(
                            "p (a b) -> p a b", a=w // 128)
                        nc.scalar.dma_start(dst, src)

                def load_w(wap, kc, chunks=1):
                    t = w_pool.tile([128, 2048], bf16, tag="w", name="wtile")
                    for c in range(chunks):
                        w = 512 // chunks
                        r0 = kc * 512 + c * w
                        src = wap[r0:r0 + w, :].rearrange(
                            "(ks p) n -> p ks n", p=128)
                        dst = t[:, c * w * 4:(c + 1) * w * 4].rearrange(
                            "p (a b) -> p a b", a=w // 128)
                        nc.scalar.dma_start(dst, src)
                    return t

                def rope_evict(ps, dst):
                    tsw = rope_pool.tile([128, NTOK], f32, tag="tsw",
                                         name="tsw")
                    nc.vector.tensor_copy(tsw[0:64, :], ps[64:128, :])
                    nc.vector.tensor_copy(tsw[64:128, :], ps[0:64, :])
                    tco = rope_pool.tile([128, NTOK], bf16, tag="tco",
                                         name="tco")
                    nc.vector.tensor_mul(tco[:], ps[:], cos_sb[:])
                    tsi = rope_pool.tile([128, NTOK], bf16, tag="tsi",
                                         name="tsi")
                    nc.vector.tensor_mul(tsi[:], tsw[:], sin_sb[:])
                    nc.vector.tensor_add(dst[:], tco[:], tsi[:])

                def mm_qk(ps, wt, ks, m, xtile, xoff, first, last):
                    # out[hd_m, tok] += w[:, m].T @ xT
                    nc.tensor.matmul(
                        ps[m][:],
                        wt[:, ks * 512 + m * 128:ks * 512 + (m + 1) * 128],
                        xtile[:, xoff:xoff + 512], start=first, stop=last)

                for wap, dsts in ((wqd, QT), (wkd, KT)):
                    ps = [pp.tile([128, 512], f32, tag=f"pp{m}",
                                  name=f"pp{m}") for m in range(4)]
                    for kc in range(8):
                        if wap is wqd and kc < 2:
                            # fine-grained first x/w loads, x on Activation
                            # and w on SP in parallel, so PE starts after ~2
                            # small DMAs and never starves in the ramp-up
                            wt = w_pool.tile([128, 2048], bf16, tag="w",
                                             name="wtile")
                            for c in range(4):
                                r = kc * 512 + c * 128
                                nc.scalar.dma_start(
                                    XT[0][:, kc * 2048 + c * 512:
                                          kc * 2048 + (c + 1) * 512],
                                    xt[r:r + 128, :])
                                nc.sync.dma_start(
                                    wt[:, c * 512:(c + 1) * 512],
                                    wqd[r:r + 128, :])
                        else:
                            if wap is wqd:
                                # interleave x halves with the w stream
                                load_x_half(kc // 2, kc % 2)
                            wt = load_w(wap, kc)
                        xtile = XT[kc // 2]
                        if kc < 7:
                            for ks in range(4):
                                first = (kc == 0 and ks == 0)
                                xoff = (kc % 2) * 2048 + ks * 512
                                for m in range(4):
                                    mm_qk(ps, wt, ks, m, xtile, xoff, first,
                                          False)
                        else:
                            # last k-chunk: m-major so each head's psum can
                            # be evicted while the next head still matmuls
                            for m in range(4):
                                for ks in range(4):
                                    xoff = (kc % 2) * 2048 + ks * 512
                                    mm_qk(ps, wt, ks, m, xtile, xoff, False,
                                          ks == 3)
                                rope_evict(ps[m], dsts[m])

                # V projection (bf16): out[tok_m, feat] += xT[:, m].T @ wv
                ps = [pp.tile([128, 512], f32, tag=f"pp{m}",
                              name=f"pp{m}") for m in range(4)]
                for kc in range(8):
                    wt = load_w(wvd, kc)
                    xtile = XT[kc // 2]
                    if kc < 7:
                        for ks in range(4):
                            xoff = (kc % 2) * 2048 + ks * 512
                            for m in range(4):
                                nc.tensor.matmul(
                                    ps[m][:],
                                    xtile[:, xoff + m * 128:
                                          xoff + (m + 1) * 128],
                                    wt[:, ks * 512:(ks + 1) * 512],
                                    start=(kc == 0 and ks == 0), stop=False)
                    else:
                        for m in range(4):
                            for ks in range(4):
                                xoff = (kc % 2) * 2048 + ks * 512
                                nc.tensor.matmul(
                                    ps[m][:],
                                    xtile[:, xoff + m * 128:
                                          xoff + (m + 1) * 128],
                                    wt[:, ks * 512:(ks + 1) * 512],
                                    start=False, stop=(ks == 3))
                            nc.vector.tensor_copy(VN[m][:], ps[m][:])

            # ------------- Phase B: attention + grouped wo ---------------
            with ExitStack() as pb:
                sp_ps = pb.enter_context(
                    tc.tile_pool(name="spps", bufs=2, space="PSUM"))
                av_ps = pb.enter_context(
                    tc.tile_pool(name="avps", bufs=2, space="PSUM"))
                po_ps = pb.enter_context(
                    tc.tile_pool(name="pops", bufs=2, space="PSUM"))

                pending = {}
                vts = {}

                # 7-deep K/V slot rings: 5 pool buffers + 2 retired XT tiles
                # (same [128, 4096] bf16 shape) per stream
                def kt_slot(b):
                    i = b % 7
                    if i < 5:
                        return kc_pool.tile([128, HL * n_cached], bf16,
                                            tag=f"kt{i}", name=f"kt{i}")
                    return XT[i - 5]

                def vt_slot(b):
                    i = b % 7
                    if i < 5:
                        return vc_pool.tile([128, HL * SCF * HD], bf16,
                                            tag=f"vt{i}", name=f"vt{i}")
                    return XT[2 + i - 5]

                def emit_vt(b):
                    vt_b = vt_slot(b)
                    eng = nc.gpsimd
                    eng.dma_start(
                        vt_b[:].rearrange("p (h s) -> p h s", h=HL),
                        vcc[b].rearrange("h p s -> p h s"))
                    vts[b] = vt_b

                kts = {}

                def emit_kt(b, eng):
                    kt_b = kt_slot(b)
                    eng.dma_start(
                        kt_b[:].rearrange("p (h s) -> p h s", h=HL),
                        ktc[b].rearrange("h p s -> p h s"))
                    kts[b] = kt_b

                def emit_b(b):
                    g, j = b // 4, b % 4
                    p0 = j * 32
                    if b not in kts:
                        emit_kt(b, nc.sync)
                    kt_b = kts.pop(b)
                    prs = []
                    # two heads per psum tile (bank-aligned halves) so one
                    # activation covers both, amortizing the access penalty
                    for hp in range(2):
                        sp = sp_ps.tile([128, 1024], f32, tag="sp",
                                        name="sp")
                        for hh in range(2):
                            h = hp * 2 + hh
                            off = hh * 512
                            qs = QT[h][:, b * 32:(b + 1) * 32]
                            for sc in range(SCF):
                                nc.tensor.matmul(
                                    sp[:, off + sc * 32:off + (sc + 1) * 32],
                                    kt_b[:, h * n_cached + sc * 128:
                                         h * n_cached + (sc + 1) * 128],
                                    qs, start=True, stop=True)
                            nc.tensor.matmul(
                                sp[p0:p0 + 32, off + SCF * 32:off + CW],
                                KT[h][:, b * 32:(b + 1) * 32], qs,
                                start=True, stop=True, tile_position=(0, p0))
                        pr = pr_pool.tile([128, 2 * CW], bf16, tag="pr",
                                          name="pr")
                        nc.scalar.activation(
                            pr[:].rearrange("p (i c) -> p i c", i=2),
                            sp[:].rearrange("p (i c) -> p i c",
                                            i=2)[:, :, 0:CW],
                            mybir.ActivationFunctionType.Exp, scale=SCALE)
                        prs.append(pr)
                    pending[b] = (prs, g, p0)

                normq = {}

                def finish(b):
                    prs, g, p0 = pending.pop(b)
                    vt_b = vts.pop(b)
                    # av cols 0:128 = A.V for 4 heads; col 128 = transposed
                    # softmax denominators (token-on-partition, free size 1
                    # per accumulation step); row 0 cols 256:384 later holds
                    # the transposed reciprocals
                    av = av_ps.tile([128, 384], f32, tag="av", name="av")
                    for h in range(HL):
                        pr = prs[h // 2]
                        co = (h % 2) * CW
                        vo = h * SCF * HD
                        for sc in range(SCF):
                            nc.tensor.matmul(
                                av[:, h * 32:(h + 1) * 32],
                                vt_b[:, vo + sc * HD:vo + (sc + 1) * HD],
                                pr[:, co + sc * 32:co + (sc + 1) * 32],
                                start=(sc == 0), stop=False)
                        nc.tensor.matmul(
                            av[:, h * 32:(h + 1) * 32],
                            VN[g][p0:p0 + 32, h * HD:(h + 1) * HD],
                            pr[p0:p0 + 32, co + SCF * 32:co + CW],
                            start=False, stop=True, tile_position=(p0, 0))
                        for sc in range(SCF):
                            nc.tensor.matmul(
                                av[h * 32:(h + 1) * 32, 128:129],
                                pr[:, co + sc * 32:co + (sc + 1) * 32],
                                ones_col[:],
                                start=(sc == 0), stop=False,
                                tile_position=(0, h * 32))
                        nc.tensor.matmul(
                            av[h * 32:(h + 1) * 32, 128:129],
                            pr[p0:p0 + 32, co + SCF * 32:co + CW],
                            ones_col[p0:p0 + 32, :],
                            start=False, stop=True,
                            tile_position=(p0, h * 32))
                    rrc = rr_pool.tile([128, 1], f32, tag="rrc", name="rrc")
                    nc.vector.reciprocal(rrc[:], av[:, 128:129])
                    normq[b] = (av, rrc)

                def norm2(b):
                    # deferred 2 iterations: transpose the reciprocals to a
                    # row, broadcast on GpSimd, fused normalize-evict on DVE
                    av, rrc = normq.pop(b)
                    nc.tensor.matmul(av[0:1, 256:384], rrc[:], IDEN[:],
                                     is_transpose=True)
                    rrs = rr_pool.tile([1, 128], f32, tag="rrs", name="rrs")
                    nc.vector.tensor_copy(rrs[:], av[0:1, 256:384])
                    rbs = rb_pool.tile([128, 128], f32, tag="rbs",
                                       name="rbs")
                    nc.gpsimd.partition_broadcast(rbs[:], rrs[:])
                    dst = ATH[:].rearrange(
                        "p (h t) -> p h t", h=HL)[:, :, b * 32:(b + 1) * 32]
                    src = av[:, 0:128].rearrange("p (h t) -> p h t", h=HL)
                    rbv = rbs[:].rearrange("p (h t) -> p h t", h=HL)
                    nc.vector.tensor_mul(dst, src, rbv)

                def emit_wo(g):
                    # last chunk of the last group in two half-tiles (own
                    # psum banks, so the bank-granular dep tracker doesn't
                    # serialize them) to shorten the final evict+DMA tail
                    chunks = [(oc * 512, (oc + 1) * 512) for oc in range(8)]
                    if g == 3:
                        chunks = chunks[:7] + [(3584, 3840), (3840, 4096)]
                    for lo, hi in chunks:
                        w = hi - lo
                        po = po_ps.tile([128, 512], f32, tag="po", name="po")
                        pool = obt_pool if g == 3 else ob_pool
                        ob = pool.tile([128, 512], bf16, tag="ob",
                                       name="ob")
                        for h in range(HL):
                            nc.tensor.matmul(
                                po[:, 0:w],
                                ATH[:, h * NTOK + g * 128:
                                    h * NTOK + (g + 1) * 128],
                                WO[h][:, lo:hi],
                                start=(h == 0), stop=(h == HL - 1))
                        nc.vector.tensor_copy(ob[:, 0:w], po[:, 0:w])
                        oq = nc.scalar if g < 2 else nc.sync
                        oq.dma_start(
                            outp[g * 128:(g + 1) * 128, lo:hi], ob[:, 0:w])

                for b in range(B):
                    if b == 5:
                        # wo weights on the SP queue after the first 5 kt
                        # transfers, before the XT-slot kt DMAs that block
                        # until the projections retire
                        for h in range(HL):
                            nc.sync.dma_start(WO[h][:],
                                              wod[h * 128:(h + 1) * 128, :])
                    emit_b(b)
                    if b >= 2:
                        norm2(b - 2)
                    if b == 0:
                        emit_vt(0)
                        emit_vt(1)
                    if b > 0:
                        # norm2() ends with the partition_broadcast on the
                        # Pool queue; emit vt after it so the broadcast is
                        # never stuck behind a cache transfer
                        finish(b - 1)
                        if b + 1 < B:
                            emit_vt(b + 1)
                    # wo for group g emitted 2 batches into group g+1 so the
                    # normalize chain of the group's last batch is long done
                    if b >= 6 and (b - 6) % 4 == 0:
                        emit_wo((b - 6) // 4)
                finish(B - 1)
                norm2(B - 2)
                norm2(B - 1)
                emit_wo(3)

    nc.compile()
    return nc


def _host_prep(x, wq, wk, wv, wo, k_cache, v_cache, n_cached):
    import ml_dtypes
    bf = ml_dtypes.bfloat16

    x = np.asarray(x, dtype=np.float32)
    wq = np.asarray(wq, dtype=np.float32)
    wk = np.asarray(wk, dtype=np.float32)
    wv = np.asarray(wv, dtype=np.float32)
    wo = np.asarray(wo, dtype=np.float32)
    k_cache = np.asarray(k_cache, dtype=np.float32)
    v_cache = np.asarray(v_cache, dtype=np.float32)

    SCF = n_cached // 128
    perm = np.concatenate([np.arange(0, HD, 2), np.arange(1, HD, 2)])

    xt = np.ascontiguousarray(x.reshape(NTOK, D).T.astype(bf))  # [D, NTOK]

    # rope tables in deinterleaved layout
    theta = (np.float32(10000.0) **
             (np.float32(-2.0) * np.arange(0, HD, 2, dtype=np.float32)
              / np.float32(HD)))                      # [64]
    freqs = np.arange(T, dtype=np.float32)[:, None] * theta[None, :]  # [T,64]
    cos_t = np.cos(freqs).astype(np.float32).T        # [64, T]
    sin_t = np.sin(freqs).astype(np.float32).T
    iden = np.ascontiguousarray(np.eye(128, dtype=np.float32))
    cos_rep = np.tile(cos_t, (1, B))                  # [64, NTOK]
    sin_rep = np.tile(sin_t, (1, B))
    cosd = np.ascontiguousarray(np.concatenate([cos_rep, cos_rep], axis=0))
    sind = np.ascontiguousarray(np.concatenate([-sin_rep, sin_rep], axis=0))

    in_maps = []
    for c in range(NC):
        hs = np.arange(c * HL, (c + 1) * HL)
        cols = (hs[:, None] * HD + perm[None, :]).reshape(-1)   # permuted q/k
        colsv = (hs[:, None] * HD + np.arange(HD)[None, :]).reshape(-1)
        wq_c = np.ascontiguousarray(wq[:, cols].astype(bf))
        wk_c = np.ascontiguousarray(wk[:, cols].astype(bf))
        wv_c = np.ascontiguousarray(wv[:, colsv].astype(bf))
        wo_c = np.ascontiguousarray(wo[colsv, :].astype(bf))
        # k cache: [b, h, hd(perm), s]
        kc_c = np.ascontiguousarray(
            k_cache[:, :n_cached][:, :, hs][:, :, :, perm]
            .transpose(0, 2, 3, 1).astype(bf))
        # v cache: [b, h, sp, sc, hd] -> flat [b, h, 128, SCF*HD]
        vc_c = np.ascontiguousarray(
            v_cache[:, :n_cached][:, :, hs]
            .reshape(B, SCF, 128, HL, HD)
            .transpose(0, 3, 2, 1, 4)
            .reshape(B, HL, 128, SCF * HD).astype(bf))
        in_maps.append({
            "xt": xt, "wqd": wq_c, "wkd": wk_c, "wvd": wv_c, "wod": wo_c,
            "ktc": kc_c, "vcc": vc_c, "cosd": cosd, "sind": sind,
            "iden": iden,
        })
    return in_maps


def kernel(x, wq, wk, wv, wo, k_cache, v_cache, start_pos):
    from concourse import bass_utils

    n_cached = int(start_pos)
    assert n_cached % 128 == 0, "kernel assumes start_pos multiple of 128"

    if _STATE.get("n_cached") != n_cached:
        _STATE["nc"] = _build(n_cached)
        _STATE["n_cached"] = n_cached
    ncb = _STATE["nc"]

    in_maps = _host_prep(x, wq, wk, wv, wo, k_cache, v_cache, n_cached)
    res = None
    for attempt in range(4):
        try:
            res = bass_utils.run_bass_kernel_spmd(ncb, in_maps,
                                                  core_ids=list(range(NC)))
            break
        except Exception:
            if attempt == 3:
                raise
            import time as _time
            _time.sleep(20 * (attempt + 1))
    out = np.zeros((NTOK, D), dtype=np.float32)
    for c in range(NC):
        out += res.results[c]["outp"].astype(np.float32)
    return np.ascontiguousarray(out).reshape(B, T, D)


# revision 75
# speedup vs baseline: 1.0112x; 1.0091x over previous
"""Trainium2 Bass kernel for multi-head attention decode step with KV cache.

Problem shapes (hardcoded): x[16,32,4096], wq/wk/wv/wo[4096,4096],
k_cache/v_cache[16,2048,32,128], start_pos=1024 (must be multiple of 128).

Sharding: tensor-parallel over the 32 heads -> 4 heads per core on 8 cores.
wq/wk/wv column-sharded, wo row-sharded; per-core partial outputs (full
[512, 4096] token-major tiles) summed on host.

Everything is streamed in bf16 (weights, x, KV caches, attention tensors);
PSUM accumulation stays fp32.

Per-core plan:
  Phase A: QT/KT = (w.T @ x.T) in [head_dim, token] layout, V in
           [token, feat] layout, via bf16 matmuls. RoPE applied on PSUM
           eviction using a deinterleaved head_dim permutation (evens then
           odds) so the rotate is a 64-partition swap.
  Phase B (b outer, h inner): per batch b all 4 heads' scoresT chunks
           (s on psum partitions), exp via ScalarE (scale=1/sqrt(128)),
           denominator via ones-vector matmuls into the same psum tile as
           the A.V results, 1/denom broadcast to 128 partitions on GpSimd
           (partition_broadcast), single fused normalize-evict multiply on
           DVE into the 4-head attnT tile. The 32 new tokens' scores/V use
           matmul tile_position to sit on partitions (b%4)*32 so V-new can
           be used as lhsT directly from the projection output (no copies).
           After each 4-batch group (128 tokens), wo is applied with the
           4 heads accumulated in PSUM, evicted bf16 and written to a
           token-major [512, 4096] output (partials summed on host).

Scheduling: K/V cache tiles use 7-deep slot rings (5 pool buffers + 2
retired x tiles each) so the streams prefetch through the wo bursts.
DMA queues: SP = first w chunks + K-cache + wo weights; Activation =
x + wq/wk/wv (all before any exp work starts) + exps + output chunks;
GpSimd = cos/sin + V-cache + partition broadcasts (vt emitted one batch
ahead of its broadcast so the normalize chain is never queued behind a
3.2us transfer).
"""

import numpy as np

B, T, D = 16, 32, 4096
HL, HD = 4, 128          # heads per core, head dim
NTOK = B * T             # 512
NC = 8

_STATE = {}


def _build(n_cached):
    import concourse.tile as tile
    from concourse import bacc, mybir
    from contextlib import ExitStack

    f32 = mybir.dt.float32
    bf16 = mybir.dt.bfloat16
    f8 = mybir.dt.float8e4

    SCF = n_cached // 128          # full cached s-chunks
    CW = SCF * 32 + 32             # scoresT free width (cached + new block)

    nc = bacc.Bacc("TRN2", target_bir_lowering=False, debug=False,
                   num_devices=NC)

    xt = nc.dram_tensor("xt", [D, NTOK], bf16, kind="ExternalInput").ap()
    wqd = nc.dram_tensor("wqd", [D, HL * HD], bf16, kind="ExternalInput").ap()
    wkd = nc.dram_tensor("wkd", [D, HL * HD], bf16, kind="ExternalInput").ap()
    wvd = nc.dram_tensor("wvd", [D, HL * HD], bf16, kind="ExternalInput").ap()
    wod = nc.dram_tensor("wod", [HL * HD, D], bf16, kind="ExternalInput").ap()
    ktc = nc.dram_tensor("ktc", [B, HL, HD, n_cached], bf16,
                         kind="ExternalInput").ap()
    vcc = nc.dram_tensor("vcc", [B, HL, 128, SCF * HD], bf16,
                         kind="ExternalInput").ap()
    iden = nc.dram_tensor("iden", [128, 128], f32,
                          kind="ExternalInput").ap()
    cosd = nc.dram_tensor("cosd", [128, NTOK], f32, kind="ExternalInput").ap()
    sind = nc.dram_tensor("sind", [128, NTOK], f32, kind="ExternalInput").ap()
    outp = nc.dram_tensor("outp", [NTOK, D], bf16,
                          kind="ExternalOutput").ap()

    SCALE = float(1.0 / np.sqrt(np.float32(HD)))

    with tile.TileContext(nc) as tc:
        with ExitStack() as outer:
            st_pool = outer.enter_context(tc.tile_pool(name="st", bufs=1))
            dyn_pool = outer.enter_context(tc.tile_pool(name="dyn", bufs=2))
            kc_pool = outer.enter_context(tc.tile_pool(name="kc", bufs=1))
            vc_pool = outer.enter_context(tc.tile_pool(name="vc", bufs=1))
            pr_pool = outer.enter_context(tc.tile_pool(name="pr", bufs=6))
            obt_pool = outer.enter_context(tc.tile_pool(name="obt", bufs=4))

            cos_sb = st_pool.tile([128, NTOK], f32, tag="cos", name="cos_sb")
            sin_sb = st_pool.tile([128, NTOK], f32, tag="sin", name="sin_sb")
            nc.gpsimd.dma_start(cos_sb[:], cosd[:])
            IDEN = st_pool.tile([128, 128], f32, tag="iden", name="IDEN")
            nc.gpsimd.dma_start(IDEN[:], iden[:])
            nc.gpsimd.dma_start(sin_sb[:], sind[:])
            ones_col = st_pool.tile([128, 1], bf16, tag="oc", name="ones_col")
            nc.vector.memset(ones_col[:], 1.0)

            WO = [st_pool.tile([128, D], bf16, tag=f"wo{h}", name=f"WO{h}")
                  for h in range(HL)]

            QT = [st_pool.tile([128, NTOK], bf16, tag=f"q{m}", name=f"QT{m}")
                  for m in range(HL)]
            KT = [st_pool.tile([128, NTOK], bf16, tag=f"k{m}", name=f"KT{m}")
                  for m in range(HL)]
            VN = [st_pool.tile([128, HL * HD], bf16, tag=f"v{m}",
                               name=f"VN{m}") for m in range(4)]
            ATH = st_pool.tile([128, HL * NTOK], bf16, tag="ath", name="ATH")
            XT = [st_pool.tile([128, 4096], bf16, tag=f"x{t4}",
                               name=f"XT{t4}") for t4 in range(4)]
            rr_pool = rb_pool = ob_pool = rope_pool = dyn_pool

            # ---------------- Phase A: projections + rope ----------------
            with ExitStack() as pa:
                w_pool = pa.enter_context(tc.tile_pool(name="w", bufs=3))
                pp = pa.enter_context(
                    tc.tile_pool(name="pp", bufs=2, space="PSUM"))

                def load_x_half(t4, hf, chunks=1):
                    for c in range(chunks):
                        w = 512 // chunks
                        r0 = t4 * 1024 + hf * 512 + c * w
                        src = xt[r0:r0 + w, :].rearrange(
                            "(ks p) n -> p ks n", p=128)
                        c0 = hf * 2048 + c * w * 4
                        dst = XT[t4][:, c0:c0 + w * 4].rearrange(
                            "p (a b) -> p a b", a=w // 128)
                        nc.scalar.dma_start(dst, src)

                def load_w(wap, kc, chunks=1):
                    t = w_pool.tile([128, 2048], bf16, tag="w", name="wtile")
                    for c in range(chunks):
                        w = 512 // chunks
                        r0 = kc * 512 + c * w
                        src = wap[r0:r0 + w, :].rearrange(
                            "(ks p) n -> p ks n", p=128)
                        dst = t[:, c * w * 4:(c + 1) * w * 4].rearrange(
                            "p (a b) -> p a b", a=w // 128)
                        nc.scalar.dma_start(dst, src)
                    return t

                def rope_evict(ps, dst):
                    tsw = rope_pool.tile([128, NTOK], f32, tag="tsw",
                                         name="tsw")
                    nc.vector.tensor_copy(tsw[0:64, :], ps[64:128, :])
                    nc.vector.tensor_copy(tsw[64:128, :], ps[0:64, :])
                    tco = rope_pool.tile([128, NTOK], bf16, tag="tco",
                                         name="tco")
                    nc.vector.tensor_mul(tco[:], ps[:], cos_sb[:])
                    tsi = rope_pool.tile([128, NTOK], bf16, tag="tsi",
                                         name="tsi")
                    nc.vector.tensor_mul(tsi[:], tsw[:], sin_sb[:])
                    nc.vector.tensor_add(dst[:], tco[:], tsi[:])

                def mm_qk(ps, wt, ks, m, xtile, xoff, first, last):
                    # out[hd_m, tok] += w[:, m].T @ xT
                    nc.tensor.matmul(
                        ps[m][:],
                        wt[:, ks * 512 + m * 128:ks * 512 + (m + 1) * 128],
                        xtile[:, xoff:xoff + 512], start=first, stop=last)

                for wap, dsts in ((wqd, QT), (wkd, KT)):
                    ps = [pp.tile([128, 512], f32, tag=f"pp{m}",
                                  name=f"pp{m}") for m in range(4)]
                    for kc in range(8):
                        if wap is wqd and kc < 2:
                            # fine-grained first x/w loads, x on Activation
                            # and w on SP in parallel, so PE starts after ~2
                            # small DMAs and never starves in the ramp-up
                            wt = w_pool.tile([128, 2048], bf16, tag="w",
                                             name="wtile")
                            for c in range(4):
                                r = kc * 512 + c * 128
                                nc.scalar.dma_start(
                                    XT[0][:, kc * 2048 + c * 512:
                                          kc * 2048 + (c + 1) * 512],
                                    xt[r:r + 128, :])
                                nc.sync.dma_start(
                                    wt[:, c * 512:(c + 1) * 512],
                                    wqd[r:r + 128, :])
                        else:
                            if wap is wqd:
                                # interleave x halves with the w stream
                                load_x_half(kc // 2, kc % 2)
                            wt = load_w(wap, kc)
                        xtile = XT[kc // 2]
                        if kc < 7:
                            for ks in range(4):
                                first = (kc == 0 and ks == 0)
                                xoff = (kc % 2) * 2048 + ks * 512
                                for m in range(4):
                                    mm_qk(ps, wt, ks, m, xtile, xoff, first,
                                          False)
                        else:
                            # last k-chunk: m-major so each head's psum can
                            # be evicted while the next head still matmuls
                            for m in range(4):
                                for ks in range(4):
                                    xoff = (kc % 2) * 2048 + ks * 512
                                    mm_qk(ps, wt, ks, m, xtile, xoff, False,
                                          ks == 3)
                                rope_evict(ps[m], dsts[m])

                # V projection (bf16): out[tok_m, feat] += xT[:, m].T @ wv
                ps = [pp.tile([128, 512], f32, tag=f"pp{m}",
                              name=f"pp{m}") for m in range(4)]
                for kc in range(8):
                    wt = load_w(wvd, kc)
                    xtile = XT[kc // 2]
                    if kc < 7:
                        for ks in range(4):
                            xoff = (kc % 2) * 2048 + ks * 512
                            for m in range(4):
                                nc.tensor.matmul(
                                    ps[m][:],
                                    xtile[:, xoff + m * 128:
                                          xoff + (m + 1) * 128],
                                    wt[:, ks * 512:(ks + 1) * 512],
                                    start=(kc == 0 and ks == 0), stop=False)
                    else:
                        for m in range(4):
                            for ks in range(4):
                                xoff = (kc % 2) * 2048 + ks * 512
                                nc.tensor.matmul(
                                    ps[m][:],
                                    xtile[:, xoff + m * 128:
                                          xoff + (m + 1) * 128],
                                    wt[:, ks * 512:(ks + 1) * 512],
                                    start=False, stop=(ks == 3))
                            nc.vector.tensor_copy(VN[m][:], ps[m][:])

            # ------------- Phase B: attention + grouped wo ---------------
            with ExitStack() as pb:
                sp_ps = pb.enter_context(
                    tc.tile_pool(name="spps", bufs=2, space="PSUM"))
                av_ps = pb.enter_context(
                    tc.tile_pool(name="avps", bufs=2, space="PSUM"))
                po_ps = pb.enter_context(
                    tc.tile_pool(name="pops", bufs=2, space="PSUM"))

                pending = {}
                vts = {}

                # 7-deep K/V slot rings: 5 pool buffers + 2 retired XT tiles
                # (same [128, 4096] bf16 shape) per stream
                def kt_slot(b):
                    i = b % 7
                    if i < 5:
                        return kc_pool.tile([128, HL * n_cached], bf16,
                                            tag=f"kt{i}", name=f"kt{i}")
                    return XT[i - 5]

                def vt_slot(b):
                    i = b % 7
                    if i < 5:
                        return vc_pool.tile([128, HL * SCF * HD], bf16,
                                            tag=f"vt{i}", name=f"vt{i}")
                    return XT[2 + i - 5]

                def emit_vt(b):
                    vt_b = vt_slot(b)
                    eng = nc.gpsimd
                    eng.dma_start(
                        vt_b[:].rearrange("p (h s) -> p h s", h=HL),
                        vcc[b].rearrange("h p s -> p h s"))
                    vts[b] = vt_b

                kts = {}

                def emit_kt(b, eng):
                    kt_b = kt_slot(b)
                    eng.dma_start(
                        kt_b[:].rearrange("p (h s) -> p h s", h=HL),
                        ktc[b].rearrange("h p s -> p h s"))
                    kts[b] = kt_b

                def emit_b(b):
                    g, j = b // 4, b % 4
                    p0 = j * 32
                    if b not in kts:
                        emit_kt(b, nc.sync)
                    kt_b = kts.pop(b)
                    prs = []
                    # two heads per psum tile (bank-aligned halves) so one
                    # activation covers both, amortizing the access penalty
                    for hp in range(2):
                        sp = sp_ps.tile([128, 1024], f32, tag="sp",
                                        name="sp")
                        for hh in range(2):
                            h = hp * 2 + hh
                            off = hh * 512
                            qs = QT[h][:, b * 32:(b + 1) * 32]
                            for sc in range(SCF):
                                nc.tensor.matmul(
                                    sp[:, off + sc * 32:off + (sc + 1) * 32],
                                    kt_b[:, h * n_cached + sc * 128:
                                         h * n_cached + (sc + 1) * 128],
                                    qs, start=True, stop=True)
                            nc.tensor.matmul(
                                sp[p0:p0 + 32, off + SCF * 32:off + CW],
                                KT[h][:, b * 32:(b + 1) * 32], qs,
                                start=True, stop=True, tile_position=(0, p0))
                        pr = pr_pool.tile([128, 2 * CW], bf16, tag="pr",
                                          name="pr")
                        nc.scalar.activation(
                            pr[:].rearrange("p (i c) -> p i c", i=2),
                            sp[:].rearrange("p (i c) -> p i c",
                                            i=2)[:, :, 0:CW],
                            mybir.ActivationFunctionType.Exp, scale=SCALE)
                        prs.append(pr)
                    pending[b] = (prs, g, p0)

                normq = {}

                def finish(b):
                    prs, g, p0 = pending.pop(b)
                    vt_b = vts.pop(b)
                    # av cols 0:128 = A.V for 4 heads; col 128 = transposed
                    # softmax denominators (token-on-partition, free size 1
                    # per accumulation step); row 0 cols 256:384 later holds
                    # the transposed reciprocals
                    av = av_ps.tile([128, 384], f32, tag="av", name="av")
                    for h in range(HL):
                        pr = prs[h // 2]
                        co = (h % 2) * CW
                        vo = h * SCF * HD
                        for sc in range(SCF):
                            nc.tensor.matmul(
                                av[:, h * 32:(h + 1) * 32],
                                vt_b[:, vo + sc * HD:vo + (sc + 1) * HD],
                                pr[:, co + sc * 32:co + (sc + 1) * 32],
                                start=(sc == 0), stop=False)
                        nc.tensor.matmul(
                            av[:, h * 32:(h + 1) * 32],
                            VN[g][p0:p0 + 32, h * HD:(h + 1) * HD],
                            pr[p0:p0 + 32, co + SCF * 32:co + CW],
                            start=False, stop=True, tile_position=(p0, 0))
                        for sc in range(SCF):
                            nc.tensor.matmul(
                                av[h * 32:(h + 1) * 32, 128:129],
                                pr[:, co + sc * 32:co + (sc + 1) * 32],
                                ones_col[:],
                                start=(sc == 0), stop=False,
                                tile_position=(0, h * 32))
                        nc.tensor.matmul(
                            av[h * 32:(h + 1) * 32, 128:129],
                            pr[p0:p0 + 32, co + SCF * 32:co + CW],
                            ones_col[p0:p0 + 32, :],
                            start=False, stop=True,
                            tile_position=(p0, h * 32))
                    rrc = rr_pool.tile([128, 1], f32, tag="rrc", name="rrc")
                    nc.vector.reciprocal(rrc[:], av[:, 128:129])
                    normq[b] = (av, rrc)

                def norm2(b):
                    # deferred 2 iterations: transpose the reciprocals to a
                    # row, broadcast on GpSimd, fused normalize-evict on DVE
                    av, rrc = normq.pop(b)
                    nc.tensor.matmul(av[0:1, 256:384], rrc[:], IDEN[:],
                                     is_transpose=True)
                    rrs = rr_pool.tile([1, 128], f32, tag="rrs", name="rrs")
                    nc.vector.tensor_copy(rrs[:], av[0:1, 256:384])
                    rbs = rb_pool.tile([128, 128], f32, tag="rbs",
                                       name="rbs")
                    nc.gpsimd.partition_broadcast(rbs[:], rrs[:])
                    dst = ATH[:].rearrange(
                        "p (h t) -> p h t", h=HL)[:, :, b * 32:(b + 1) * 32]
                    src = av[:, 0:128].rearrange("p (h t) -> p h t", h=HL)
                    rbv = rbs[:].rearrange("p (h t) -> p h t", h=HL)
                    nc.vector.tensor_mul(dst, src, rbv)

                def emit_wo(g, ocs=None):
                    # last chunk of the last group in two half-tiles (own
                    # psum banks, so the bank-granular dep tracker doesn't
                    # serialize them) to shorten the final evict+DMA tail
                    chunks = [(oc * 512, (oc + 1) * 512) for oc in range(8)]
                    if g == 3:
                        chunks = chunks[:7] + [(3584, 3840), (3840, 4096)]
                    if ocs is not None:
                        chunks = [chunks[i] for i in ocs]
                    for lo, hi in chunks:
                        w = hi - lo
                        po = po_ps.tile([128, 512], f32, tag="po", name="po")
                        pool = obt_pool if g == 3 else ob_pool
                        ob = pool.tile([128, 512], bf16, tag="ob",
                                       name="ob")
                        for h in range(HL):
                            nc.tensor.matmul(
                                po[:, 0:w],
                                ATH[:, h * NTOK + g * 128:
                                    h * NTOK + (g + 1) * 128],
                                WO[h][:, lo:hi],
                                start=(h == 0), stop=(h == HL - 1))
                        nc.vector.tensor_copy(ob[:, 0:w], po[:, 0:w])
                        oq = nc.scalar if g < 2 else nc.sync
                        oq.dma_start(
                            outp[g * 128:(g + 1) * 128, lo:hi], ob[:, 0:w])

                for b in range(B):
                    if b == 5:
                        # wo weights on the SP queue after the first 5 kt
                        # transfers, before the XT-slot kt DMAs that block
                        # until the projections retire
                        for h in range(HL):
                            nc.sync.dma_start(WO[h][:],
                                              wod[h * 128:(h + 1) * 128, :])
                    emit_b(b)
                    if b >= 2:
                        norm2(b - 2)
                    if b == 0:
                        emit_vt(0)
                        emit_vt(1)
                    if b > 0:
                        # norm2() ends with the partition_broadcast on the
                        # Pool queue; emit vt after it so the broadcast is
                        # never stuck behind a cache transfer
                        finish(b - 1)
                        if b + 1 < B:
                            emit_vt(b + 1)
                    # wo for group g emitted 2 batches into group g+1 so the
                    # normalize chain of the group's last batch is long done
                    # g0/g1 wo chunks spread 2-per-iteration over the next
                    # four iterations (smooths the DVE evict bursts and fills
                    # PE supply-wait gaps); g2 stays a burst at b=14
                    if 6 <= b <= 9:
                        emit_wo(0, ocs=[2 * (b - 6), 2 * (b - 6) + 1])
                    elif 10 <= b <= 13:
                        emit_wo(1, ocs=[2 * (b - 10), 2 * (b - 10) + 1])
                    elif b == 14:
                        emit_wo(2, ocs=[0, 1, 2, 3])
                    elif b == 15:
                        emit_wo(2, ocs=[4, 5, 6, 7])
                finish(B - 1)
                norm2(B - 2)
                norm2(B - 1)
                emit_wo(3)

    nc.compile()
    return nc


def _host_prep(x, wq, wk, wv, wo, k_cache, v_cache, n_cached):
    import ml_dtypes
    bf = ml_dtypes.bfloat16

    x = np.asarray(x, dtype=np.float32)
    wq = np.asarray(wq, dtype=np.float32)
    wk = np.asarray(wk, dtype=np.float32)
    wv = np.asarray(wv, dtype=np.float32)
    wo = np.asarray(wo, dtype=np.float32)
    k_cache = np.asarray(k_cache, dtype=np.float32)
    v_cache = np.asarray(v_cache, dtype=np.float32)

    SCF = n_cached // 128
    perm = np.concatenate([np.arange(0, HD, 2), np.arange(1, HD, 2)])

    xt = np.ascontiguousarray(x.reshape(NTOK, D).T.astype(bf))  # [D, NTOK]

    # rope tables in deinterleaved layout
    theta = (np.float32(10000.0) **
             (np.float32(-2.0) * np.arange(0, HD, 2, dtype=np.float32)
              / np.float32(HD)))                      # [64]
    freqs = np.arange(T, dtype=np.float32)[:, None] * theta[None, :]  # [T,64]
    cos_t = np.cos(freqs).astype(np.float32).T        # [64, T]
    sin_t = np.sin(freqs).astype(np.float32).T
    iden = np.ascontiguousarray(np.eye(128, dtype=np.float32))
    cos_rep = np.tile(cos_t, (1, B))                  # [64, NTOK]
    sin_rep = np.tile(sin_t, (1, B))
    cosd = np.ascontiguousarray(np.concatenate([cos_rep, cos_rep], axis=0))
    sind = np.ascontiguousarray(np.concatenate([-sin_rep, sin_rep], axis=0))

    in_maps = []
    for c in range(NC):
        hs = np.arange(c * HL, (c + 1) * HL)
        cols = (hs[:, None] * HD + perm[None, :]).reshape(-1)   # permuted q/k
        colsv = (hs[:, None] * HD + np.arange(HD)[None, :]).reshape(-1)
        wq_c = np.ascontiguousarray(wq[:, cols].astype(bf))
        wk_c = np.ascontiguousarray(wk[:, cols].astype(bf))
        wv_c = np.ascontiguousarray(wv[:, colsv].astype(bf))
        wo_c = np.ascontiguousarray(wo[colsv, :].astype(bf))
        # k cache: [b, h, hd(perm), s]
        kc_c = np.ascontiguousarray(
            k_cache[:, :n_cached][:, :, hs][:, :, :, perm]
            .transpose(0, 2, 3, 1).astype(bf))
        # v cache: [b, h, sp, sc, hd] -> flat [b, h, 128, SCF*HD]
        vc_c = np.ascontiguousarray(
            v_cache[:, :n_cached][:, :, hs]
            .reshape(B, SCF, 128, HL, HD)
            .transpose(0, 3, 2, 1, 4)
            .reshape(B, HL, 128, SCF * HD).astype(bf))
        in_maps.append({
            "xt": xt, "wqd": wq_c, "wkd": wk_c, "wvd": wv_c, "wod": wo_c,
            "ktc": kc_c, "vcc": vc_c, "cosd": cosd, "sind": sind,
            "iden": iden,
        })
    return in_maps


def kernel(x, wq, wk, wv, wo, k_cache, v_cache, start_pos):
    from concourse import bass_utils

    n_cached = int(start_pos)
    assert n_cached % 128 == 0, "kernel assumes start_pos multiple of 128"

    if _STATE.get("n_cached") != n_cached:
        _STATE["nc"] = _build(n_cached)
        _STATE["n_cached"] = n_cached
    ncb = _STATE["nc"]

    in_maps = _host_prep(x, wq, wk, wv, wo, k_cache, v_cache, n_cached)
    res = None
    for attempt in range(4):
        try:
            res = bass_utils.run_bass_kernel_spmd(ncb, in_maps,
                                                  core_ids=list(range(NC)))
            break
        except Exception:
            if attempt == 3:
                raise
            import time as _time
            _time.sleep(20 * (attempt + 1))
    out = np.zeros((NTOK, D), dtype=np.float32)
    for c in range(NC):
        out += res.results[c]["outp"].astype(np.float32)
    return np.ascontiguousarray(out).reshape(B, T, D)
